# revision 19
# baseline (speedup 1.0000x reference)
"""CGGR loss kernel for 8 TRN2 NeuronCores.

Strategy (data-parallel over the flattened token axis):
  - Each core gets 512 of the 4096 token rows (full vocab, f32).
  - On-device streaming pass over the (512, 50257) shard:
      * DVE tensor_scalar (copy->bf16) with per-1024-column max accum
        -> exact f32 chunk maxes (50 per token) + bf16 logits for pass 3
      * ACT exp (bf16 out) with sum accum -> sum(exp(l)) partials
      * DVE scalar_tensor_tensor e*l with sum accum -> sum(exp(l)*l) partials
  - Host epilogue (O(N) + one 1024-wide window gather per token):
      exact top-2 logits from chunk maxes + argmax-chunk window rescan,
      logsumexp / CE loss / entropy / margin / difficulty, global top-k
      threshold, masked mean.
"""

import numpy as np

B, S, V = 2, 2048, 50257
N = B * S                    # 4096 tokens
NCORES = 8
TPC = N // NCORES            # 512 tokens per core
P = 128
NPT = TPC // P               # 4 partition tiles per core
DMA_F = 4096                 # vocab elems per DMA chunk
NDC = (V + DMA_F - 1) // DMA_F          # 13 DMA chunks (12 full + 1105)
MAXC = 2048                  # chunk-max granularity
NMC = (V + MAXC - 1) // MAXC            # 25 max chunks (24 full + 1105)
OUTW = 10 * NDC              # 130 output stats per token (8*13 top8 | 13 se | 13 sx)

MIN_TOKENS_RATIO = 0.25
WARMUP_STEPS = 1000
THRESHOLD_SENSITIVITY = 0.5

# delta variant: chunks [0, H_EXACT) use fused STT for sum(e*l); the rest
# use a second ACT exp pass at scale (1+DELTA) and finite-difference on host.
H_EXACT = 4
DELTA = 4e-3

_compiled = None

# v2 config: compute-chunk width = V2_DMA_F * V2_CG; STT on V2_STT_NUM of the
# NPT * ceil(V / (V2_DMA_F*V2_CG)) (pt, chunk) slots, FD-exp on the rest.
V2_DMA_F = 4096
V2_CG = 2
V2_LP_BUFS = 4
V2_OB = 2
V2_STT_NUM = 12
V2_INPLACE = 0
LAST_EXEC_NS = None


def _v2_layout(dma_f=V2_DMA_F, cg=V2_CG):
    cw = dma_f * cg
    ncc = (V + cw - 1) // cw
    return cw, ncc, 10 * ncc


def _v2_stt_flags(ncc, stt_num):
    total = NPT * ncc
    return [
        ((i + 1) * stt_num) // total - (i * stt_num) // total == 1
        for i in range(total)
    ]


def _build2(reps=1, dma_f=V2_DMA_F, cg=V2_CG, lp_bufs=V2_LP_BUFS, ob=V2_OB,
            stt_num=V2_STT_NUM, inplace=V2_INPLACE):
    """Balanced v2: per compute chunk (cg DMA chunks wide):
    ACT exp->bf16 (+se accum), DVE Max8 on l (exact top-8), and either
    DVE STT e*l (+sx accum) or ACT exp@(1+DELTA) (+sx accum, host FD).
    inplace: STT/FD overwrite the (dead) e tile instead of a scratch pool."""
    import concourse.bacc as bacc
    import concourse.tile as tile
    import concourse.mybir as mybir
    from contextlib import ExitStack

    nc = bacc.Bacc("TRN2", target_bir_lowering=False, debug=False,
                   num_devices=NCORES)
    f32 = mybir.dt.float32
    bf16 = mybir.dt.bfloat16
    cw, ncc, outw = _v2_layout(dma_f, cg)
    logits = nc.dram_tensor("logits", [TPC, V], f32, kind="ExternalInput")
    out = nc.dram_tensor("out", [NPT, P, outw], f32, kind="ExternalOutput")
    stt_flags = _v2_stt_flags(ncc, stt_num)

    with tile.TileContext(nc) as tc:
        with ExitStack() as ctx:
            lp = ctx.enter_context(tc.tile_pool(name="lp", bufs=lp_bufs))
            ep = ctx.enter_context(tc.tile_pool(name="ep", bufs=ob))
            sp = ep if inplace else ctx.enter_context(
                tc.tile_pool(name="sp", bufs=ob))
            accp = ctx.enter_context(tc.tile_pool(name="accp", bufs=2))
            for rep in range(reps):
              for pt in range(NPT):
                acc = accp.tile([P, outw], f32, tag="acc")
                r0 = pt * P
                for c in range(ncc):
                    w = min(cw, V - c * cw)
                    l = lp.tile([P, cw], f32, tag="l")
                    o = 0
                    while o < w:
                        dw = min(dma_f, w - o)
                        col = c * cw + o
                        nc.sync.dma_start(
                            l[:, o:o + dw],
                            logits[r0:r0 + P, col:col + dw])
                        o += dw
                    e = ep.tile([P, cw], bf16, tag="e")
                    nc.scalar.activation(
                        out=e[:, :w], in_=l[:, :w],
                        func=mybir.ActivationFunctionType.Exp,
                        accum_out=acc[:, 8 * ncc + c:8 * ncc + c + 1])
                    nc.vector.max(out=acc[:, c * 8:(c + 1) * 8], in_=l[:, :w])
                    scr = e if inplace else sp.tile([P, cw], bf16, tag="scr")
                    sacc = acc[:, 9 * ncc + c:9 * ncc + c + 1]
                    if stt_flags[pt * ncc + c]:
                        nc.vector.scalar_tensor_tensor(
                            out=scr[:, :w], in0=e[:, :w], scalar=1.0,
                            in1=l[:, :w], op0=mybir.AluOpType.mult,
                            op1=mybir.AluOpType.mult, accum_out=sacc)
                    else:
                        nc.scalar.activation(
                            out=scr[:, :w], in_=l[:, :w],
                            func=mybir.ActivationFunctionType.Exp,
                            scale=1.0 + DELTA, accum_out=sacc)
                nc.sync.dma_start(out[pt], acc[:])
    nc.compile()
    return nc


def _v3_stt_flags(ncc, stt_num, mode="spread"):
    """STT assignment over the NPT*(ncc-1) full-width slots; the remainder
    chunk (c == ncc-1) is always FD. mode: spread|early|late placement."""
    nf = ncc - 1
    total = NPT * nf
    if mode == "spread":
        fl = [
            ((i + 1) * stt_num) // total - (i * stt_num) // total == 1
            for i in range(total)
        ]
    else:
        per = [stt_num // NPT + (1 if p < stt_num % NPT else 0)
               for p in range(NPT)]
        fl = []
        for p in range(NPT):
            if mode == "early":
                fl += [c < per[p] for c in range(nf)]
            else:
                fl += [c >= nf - per[p] for c in range(nf)]
    out = []
    for pt in range(NPT):
        for c in range(ncc):
            out.append(fl[pt * nf + c] if c < nf - 0 else False)
    return out


def _build3(reps=1, dma_f=V2_DMA_F, cg=V2_CG, lp_bufs=5, ob=2,
            stt_num=13, fd_psum=1, stt_mode="spread", dma_mix=0):
    """v3: exp->e bf16 (+se); chunk max via bf16 TT-max halving tree on e
    (2x rate) + Max8 finisher; sel via STT e*l in-place (stt slots) or
    ACT exp@(1+DELTA) into a PSUM scratch (fd slots, host FD)."""
    import concourse.bacc as bacc
    import concourse.tile as tile
    import concourse.mybir as mybir
    from contextlib import ExitStack

    nc = bacc.Bacc("TRN2", target_bir_lowering=False, debug=False,
                   num_devices=NCORES)
    f32 = mybir.dt.float32
    bf16 = mybir.dt.bfloat16
    Alu = mybir.AluOpType
    cw, ncc, outw = _v2_layout(dma_f, cg)
    assert cw == 8192
    logits = nc.dram_tensor("logits", [TPC, V], f32, kind="ExternalInput")
    out = nc.dram_tensor("out", [NPT, P, outw], f32, kind="ExternalOutput")
    stt_flags = _v3_stt_flags(ncc, stt_num, stt_mode)

    with tile.TileContext(nc) as tc:
        with ExitStack() as ctx:
            lp = ctx.enter_context(tc.tile_pool(name="lp", bufs=lp_bufs))
            ep = ctx.enter_context(tc.tile_pool(name="ep", bufs=ob))
            trp = ctx.enter_context(tc.tile_pool(name="trp", bufs=1))
            fdp = None if fd_psum else ctx.enter_context(
                tc.tile_pool(name="fdp", bufs=ob))
            accp = ctx.enter_context(tc.tile_pool(name="accp", bufs=2))
            for rep in range(reps):
              for pt in range(NPT):
                acc = accp.tile([P, outw], f32, tag="acc")
                r0 = pt * P
                for c in range(ncc):
                    w = min(cw, V - c * cw)
                    full = w == cw
                    l = lp.tile([P, cw], f32, tag="l")
                    if dma_mix == 3:
                        deng = [nc.sync, nc.scalar][(pt * ncc + c) % 2]
                        deng.dma_start(l[:, :w], logits[r0:r0 + P,
                                                        c * cw:c * cw + w])
                    else:
                        o = 0
                        di = 0
                        while o < w:
                            dw = min(dma_f, w - o)
                            col = c * cw + o
                            if dma_mix == 1:
                                deng = [nc.sync, nc.gpsimd][di % 2]
                            elif dma_mix == 2:
                                deng = [nc.sync, nc.scalar][di % 2]
                            else:
                                deng = nc.sync
                            deng.dma_start(
                                l[:, o:o + dw],
                                logits[r0:r0 + P, col:col + dw])
                            o += dw
                            di += 1
                    e = ep.tile([P, cw], bf16, tag="e")
                    nc.scalar.activation(
                        out=e[:, :w], in_=l[:, :w],
                        func=mybir.ActivationFunctionType.Exp,
                        accum_out=acc[:, 8 * ncc + c:8 * ncc + c + 1])
                    m8 = acc[:, c * 8:(c + 1) * 8]
                    sacc = acc[:, 9 * ncc + c:9 * ncc + c + 1]
                    stt = stt_flags[pt * ncc + c]
                    if full:
                        tr = trp.tile([P, cw // 2], bf16, tag="tr")
                        nc.vector.tensor_tensor(
                            out=tr[:, :4096], in0=e[:, :4096],
                            in1=e[:, 4096:8192], op=Alu.max)
                        if stt:
                            nc.vector.scalar_tensor_tensor(
                                out=e[:, :w], in0=e[:, :w], scalar=1.0,
                                in1=l[:, :w], op0=Alu.mult, op1=Alu.mult,
                                accum_out=sacc)
                        k = 2048
                        while k >= 512:
                            nc.vector.tensor_tensor(
                                out=tr[:, :k], in0=tr[:, :k],
                                in1=tr[:, k:2 * k], op=Alu.max)
                            k //= 2
                        nc.vector.max(out=m8, in_=tr[:, :512])
                    else:
                        nc.vector.max(out=m8, in_=e[:, :w])
                    if not stt:
                        # fd_psum mode: exp@(1+DELTA) written in-place over
                        # the dead l tile (f32); else a bf16 SBUF scratch.
                        scr = l if fd_psum else fdp.tile(
                            [P, cw], bf16, tag="fscr")
                        nc.scalar.activation(
                            out=scr[:, :w], in_=l[:, :w],
                            func=mybir.ActivationFunctionType.Exp,
                            scale=1.0 + DELTA, accum_out=sacc)
                nc.sync.dma_start(out[pt], acc[:])
    nc.compile()
    return nc


def _build(reps=1, variant="ttsplit", dma_f=DMA_F, lp_bufs=3, maxc=MAXC, h_exact=H_EXACT, ob=2):
    if variant.startswith("v3"):
        mode = {"v3e": "early", "v3l": "late"}.get(variant[:3] if len(variant) < 4 else variant, "spread")
        if variant in ("v3e", "v3l"):
            mode = {"v3e": "early", "v3l": "late"}[variant]
        else:
            mode = "spread"
        return _build3(reps=reps, dma_f=dma_f, cg=maxc, lp_bufs=lp_bufs,
                       ob=ob, stt_num=h_exact, stt_mode=mode,
                       fd_psum=0 if variant == "v3s" else 1,
                       dma_mix={"v3g": 1, "v3d2": 2, "v3d1": 3}.get(variant, 0))
    if variant in ("v2", "v2i"):
        return _build2(reps=reps, dma_f=dma_f, cg=maxc, lp_bufs=lp_bufs,
                       ob=ob, stt_num=h_exact, inplace=variant == "v2i")
    import concourse.bacc as bacc
    import concourse.tile as tile
    import concourse.mybir as mybir

    nc = bacc.Bacc("TRN2", target_bir_lowering=False, debug=False,
                   num_devices=NCORES)
    f32 = mybir.dt.float32
    bf16 = mybir.dt.bfloat16
    logits = nc.dram_tensor("logits", [TPC, V], f32, kind="ExternalInput")
    out = nc.dram_tensor("out", [NPT, P, OUTW], f32, kind="ExternalOutput")

    if variant.startswith("mi_"):
        return _build_micro(nc, tile, mybir, reps, variant, logits, out)
    ndc = (V + dma_f - 1) // dma_f
    with tile.TileContext(nc) as tc:
        with (
            tc.tile_pool(name="lp", bufs=lp_bufs) as lp,
            tc.tile_pool(name="lbp", bufs=ob) as lbp,
            tc.tile_pool(name="ep", bufs=ob) as ep,
            tc.tile_pool(name="sp", bufs=ob) as sp,
            tc.tile_pool(name="accp", bufs=2) as accp,
        ):
            for rep in range(reps):
              for pt in range(NPT):
                if variant == "delta3":
                    acc_m8 = accp.tile([P, 8 * ndc], f32, tag="acc_m8")
                    acc_se = accp.tile([P, ndc], f32, tag="acc_se")
                    acc_sx = accp.tile([P, ndc], f32, tag="acc_sx")
                    for dc in range(ndc):
                        w = min(dma_f, V - dc * dma_f)
                        l = lp.tile([P, dma_f], f32)
                        nc.sync.dma_start(
                            l[:, :w],
                            logits[pt * P:(pt + 1) * P,
                                   dc * dma_f:dc * dma_f + w],
                        )
                        nc.vector.max(
                            out=acc_m8[:, dc * 8:(dc + 1) * 8],
                            in_=l[:, :w])
                        e = ep.tile([P, dma_f], bf16)
                        nc.scalar.activation(
                            out=e[:, :w], in_=l[:, :w],
                            func=mybir.ActivationFunctionType.Exp,
                            accum_out=acc_se[:, dc:dc + 1],
                        )
                        scr = sp.tile([P, dma_f], bf16)
                        if dc < h_exact:
                            nc.vector.scalar_tensor_tensor(
                                out=scr[:, :w], in0=e[:, :w], scalar=1.0,
                                in1=l[:, :w],
                                op0=mybir.AluOpType.mult,
                                op1=mybir.AluOpType.mult,
                                accum_out=acc_sx[:, dc:dc + 1],
                            )
                        else:
                            nc.scalar.activation(
                                out=scr[:, :w], in_=l[:, :w],
                                func=mybir.ActivationFunctionType.Exp,
                                scale=1.0 + DELTA,
                                accum_out=acc_sx[:, dc:dc + 1],
                            )
                    nc.sync.dma_start(out[pt, :, 0:8 * ndc], acc_m8[:])
                    nc.sync.dma_start(
                        out[pt, :, 8 * NDC:8 * NDC + ndc], acc_se[:])
                    nc.sync.dma_start(
                        out[pt, :, 9 * NDC:9 * NDC + ndc], acc_sx[:])
                    continue
                if variant == "delta2":
                    nmc_l = (V + maxc - 1) // maxc
                    acc_mc = accp.tile([P, nmc_l], f32, tag="acc_mc")
                    acc_se = accp.tile([P, ndc], f32, tag="acc_se")
                    acc_sx = accp.tile([P, ndc], f32, tag="acc_sx")
                    for dc in range(ndc):
                        w = min(dma_f, V - dc * dma_f)
                        l = lp.tile([P, dma_f], f32)
                        nc.sync.dma_start(
                            l[:, :w],
                            logits[pt * P:(pt + 1) * P,
                                   dc * dma_f:dc * dma_f + w],
                        )
                        base = dc * dma_f
                        o = 0
                        while o < w:
                            cw = min(maxc, w - o)
                            mci = (base + o) // maxc
                            scrm = lbp.tile([P, dma_f], bf16, tag="scrm")
                            nc.vector.tensor_scalar(
                                out=scrm[:, :cw], in0=l[:, o:o + cw],
                                scalar1=0.0, scalar2=None,
                                op0=mybir.AluOpType.add,
                                op1=mybir.AluOpType.max,
                                accum_out=acc_mc[:, mci:mci + 1],
                            )
                            o += cw
                        e = ep.tile([P, dma_f], bf16)
                        nc.scalar.activation(
                            out=e[:, :w], in_=l[:, :w],
                            func=mybir.ActivationFunctionType.Exp,
                            accum_out=acc_se[:, dc:dc + 1],
                        )
                        scr = sp.tile([P, dma_f], bf16)
                        if dc < h_exact:
                            nc.vector.scalar_tensor_tensor(
                                out=scr[:, :w], in0=e[:, :w], scalar=1.0,
                                in1=l[:, :w],
                                op0=mybir.AluOpType.mult,
                                op1=mybir.AluOpType.mult,
                                accum_out=acc_sx[:, dc:dc + 1],
                            )
                        else:
                            nc.scalar.activation(
                                out=scr[:, :w], in_=l[:, :w],
                                func=mybir.ActivationFunctionType.Exp,
                                scale=1.0 + DELTA,
                                accum_out=acc_sx[:, dc:dc + 1],
                            )
                    nc.sync.dma_start(out[pt, :, 0:nmc_l], acc_mc[:])
                    nc.sync.dma_start(
                        out[pt, :, NMC:NMC + ndc], acc_se[:])
                    nc.sync.dma_start(
                        out[pt, :, NMC + NDC:NMC + NDC + ndc], acc_sx[:])
                    continue
                acc = accp.tile([P, OUTW], f32)
                for dc in range(ndc):
                    w = min(dma_f, V - dc * dma_f)
                    l = lp.tile([P, dma_f], f32)
                    if variant == "dmaq2":
                        deng = [nc.sync, nc.scalar][dc % 2]
                    elif variant == "dmaq3":
                        deng = [nc.sync, nc.scalar, nc.gpsimd][dc % 3]
                    elif variant == "dmaqg":
                        deng = [nc.sync, nc.gpsimd][dc % 2]
                    else:
                        deng = nc.sync
                    deng.dma_start(
                        l[:, :w],
                        logits[pt * P:(pt + 1) * P, dc * dma_f:dc * dma_f + w],
                    )
                    lb = lbp.tile([P, dma_f], bf16)
                    # per-1024 max accums (exact f32) + bf16 copy
                    pass1_eng = nc.gpsimd if variant == "tsg" else nc.vector
                    base = dc * dma_f
                    o = 0
                    while o < w:
                        cw = min(maxc, w - o)
                        mci = (base + o) // maxc
                        pass1_eng.tensor_scalar(
                            out=lb[:, o:o + cw], in0=l[:, o:o + cw],
                            scalar1=0.0, scalar2=None,
                            op0=mybir.AluOpType.add, op1=mybir.AluOpType.max,
                            accum_out=acc[:, mci:mci + 1],
                        )
                        o += cw
                        if variant.startswith("dma"):
                            break  # only one small TS per chunk (keeps DMA live)
                    if variant.startswith("dma") or variant == "nosctt_noact":
                        continue
                    if variant == "delta":
                        e = ep.tile([P, dma_f], bf16)
                        nc.scalar.activation(
                            out=e[:, :w], in_=l[:, :w],
                            func=mybir.ActivationFunctionType.Exp,
                            accum_out=acc[:, NMC + dc:NMC + dc + 1],
                        )
                        if dc < H_EXACT:
                            scr = sp.tile([P, dma_f], bf16)
                            nc.vector.scalar_tensor_tensor(
                                out=scr[:, :w], in0=e[:, :w], scalar=1.0,
                                in1=l[:, :w],
                                op0=mybir.AluOpType.mult,
                                op1=mybir.AluOpType.mult,
                                accum_out=acc[:, NMC + NDC + dc:
                                              NMC + NDC + dc + 1],
                            )
                        else:
                            scr = sp.tile([P, dma_f], bf16)
                            nc.scalar.activation(
                                out=scr[:, :w], in_=l[:, :w],
                                func=mybir.ActivationFunctionType.Exp,
                                scale=1.0 + DELTA,
                                accum_out=acc[:, NMC + NDC + dc:
                                              NMC + NDC + dc + 1],
                            )
                        continue
                    e_dt = mybir.dt.float32 if variant == "sttf32" else bf16
                    e = ep.tile([P, dma_f], e_dt)
                    nc.scalar.activation(
                        out=e[:, :w], in_=l[:, :w],
                        func=mybir.ActivationFunctionType.Exp,
                        accum_out=acc[:, NMC + dc:NMC + dc + 1],
                    )
                    if variant == "nostt":
                        continue
                    scr = sp.tile([P, dma_f], e_dt)
                    sacc = acc[:, NMC + NDC + dc:NMC + NDC + dc + 1]
                    if variant == "sttg":
                        nc.gpsimd.scalar_tensor_tensor(
                            out=scr[:, :w], in0=e[:, :w], scalar=1.0,
                            in1=lb[:, :w],
                            op0=mybir.AluOpType.mult, op1=mybir.AluOpType.mult,
                            accum_out=sacc,
                        )
                    elif variant == "ttr":
                        nc.vector.tensor_tensor_reduce(
                            out=scr[:, :w], in0=e[:, :w], in1=lb[:, :w],
                            scale=1.0, scalar=0.0,
                            op0=mybir.AluOpType.mult, op1=mybir.AluOpType.add,
                            accum_out=sacc,
                        )
                    elif variant == "amr":
                        nc.vector.affine_mul_reduce(
                            out=scr[:, :w], accum_out=sacc,
                            in0=e[:, :w], in1=lb[:, :w], scale=1.0, bias=0.0,
                        )
                    elif variant == "ttsplit":
                        nc.vector.tensor_tensor(
                            out=scr[:, :w], in0=e[:, :w], in1=lb[:, :w],
                            op=mybir.AluOpType.mult,
                        )
                        nc.vector.tensor_scalar(
                            out=scr[:, :w], in0=scr[:, :w],
                            scalar1=0.0, scalar2=None,
                            op0=mybir.AluOpType.add, op1=mybir.AluOpType.add,
                            accum_out=sacc,
                        )
                    elif variant == "tsg":
                        nc.vector.scalar_tensor_tensor(
                            out=scr[:, :w], in0=e[:, :w], scalar=1.0,
                            in1=lb[:, :w],
                            op0=mybir.AluOpType.mult, op1=mybir.AluOpType.mult,
                            accum_out=sacc,
                        )
                    elif variant == "sttf32":
                        nc.vector.scalar_tensor_tensor(
                            out=scr[:, :w], in0=e[:, :w], scalar=1.0,
                            in1=l[:, :w],
                            op0=mybir.AluOpType.mult, op1=mybir.AluOpType.mult,
                            accum_out=sacc,
                        )
                    else:
                        nc.vector.scalar_tensor_tensor(
                            out=scr[:, :w], in0=e[:, :w], scalar=1.0,
                            in1=lb[:, :w],
                            op0=mybir.AluOpType.mult, op1=mybir.AluOpType.mult,
                            accum_out=sacc,
                        )
                nc.sync.dma_start(out[pt], acc[:])

    nc.compile()
    return nc


def _build_micro(nc, tile, mybir, reps, variant, logits, out):
    """Compute-only microbench: per rep, 4 ops of FD 16384 on resident tiles."""
    f32 = mybir.dt.float32
    bf16 = mybir.dt.bfloat16
    FD = 16384
    with tile.TileContext(nc) as tc:
        with tc.tile_pool(name="mp", bufs=1) as mp:
            l = mp.tile([P, FD], f32)
            nc.sync.dma_start(l[:], logits[0:P, 0:FD])
            lb = mp.tile([P, FD], bf16)
            e = mp.tile([P, FD], bf16)
            nc.vector.tensor_scalar(out=lb[:], in0=l[:], scalar1=0.0,
                                    scalar2=None, op0=mybir.AluOpType.add)
            nc.vector.tensor_scalar(out=e[:], in0=l[:], scalar1=0.0,
                                    scalar2=None, op0=mybir.AluOpType.add)
            acc = mp.tile([P, 8], f32)
            nc.vector.memset(acc[:], 0.0)
            for rep in range(reps):
                for j in range(4):
                    a = acc[:, j:j + 1]
                    if variant == "mi_ts_max_acc":
                        nc.vector.tensor_scalar(
                            out=lb[:], in0=l[:], scalar1=0.0, scalar2=None,
                            op0=mybir.AluOpType.add, op1=mybir.AluOpType.max,
                            accum_out=a)
                    elif variant == "mi_gp_ts_max_acc":
                        nc.gpsimd.tensor_scalar(
                            out=lb[:], in0=l[:], scalar1=0.0, scalar2=None,
                            op0=mybir.AluOpType.add, op1=mybir.AluOpType.max,
                            accum_out=a)
                    elif variant == "mi_gp_reduce_max":
                        nc.gpsimd.tensor_reduce(
                            out=a, in_=l[:], op=mybir.AluOpType.max,
                            axis=mybir.AxisListType.X)
                    elif variant == "mi_dve_reduce_max":
                        nc.vector.tensor_reduce(
                            out=a, in_=l[:], op=mybir.AluOpType.max,
                            axis=mybir.AxisListType.X)
                    elif variant == "mi_max8":
                        m8 = acc[:, 0:8]
                        nc.vector.max(out=m8, in_=l[:])
                    elif variant == "mi_ts_noacc":
                        nc.vector.tensor_scalar(
                            out=lb[:], in0=l[:], scalar1=0.0, scalar2=None,
                            op0=mybir.AluOpType.add)
                    elif variant == "mi_tt_mult":
                        nc.vector.tensor_tensor(
                            out=e[:], in0=e[:], in1=lb[:],
                            op=mybir.AluOpType.mult)
                    elif variant == "mi_ts_sum_acc":
                        nc.vector.tensor_scalar(
                            out=e[:], in0=e[:], scalar1=0.0, scalar2=None,
                            op0=mybir.AluOpType.add, op1=mybir.AluOpType.add,
                            accum_out=a)
                    elif variant == "mi_ts_sum_scr":
                        nc.vector.tensor_scalar(
                            out=lb[:], in0=e[:], scalar1=0.0, scalar2=None,
                            op0=mybir.AluOpType.add, op1=mybir.AluOpType.add,
                            accum_out=a)
                    elif variant == "mi_stt":
                        nc.vector.scalar_tensor_tensor(
                            out=e[:], in0=e[:], scalar=1.0, in1=lb[:],
                            op0=mybir.AluOpType.mult,
                            op1=mybir.AluOpType.mult, accum_out=a)
                    else:
                        raise ValueError(variant)
            nc.sync.dma_start(out[0, 0:P, 0:8], acc[:])
    nc.compile()
    return nc


V3_STT_NUM = 13
V3_STT_MODE = "spread"


def _get_compiled():
    global _compiled
    if _compiled is None:
        _compiled = _build3(stt_num=V3_STT_NUM, stt_mode=V3_STT_MODE)
    return _compiled


def _device_stats(flat_logits, outw):
    """Run the bass kernel on 8 cores; return (N, outw) f32 stats."""
    global LAST_EXEC_NS
    from concourse.bass_utils import run_bass_kernel_spmd

    nc = _get_compiled()
    in_maps = [
        {"logits": np.ascontiguousarray(flat_logits[i * TPC:(i + 1) * TPC])}
        for i in range(NCORES)
    ]
    res = run_bass_kernel_spmd(nc, in_maps, list(range(NCORES)))
    LAST_EXEC_NS = res.exec_time_ns
    return np.concatenate(
        [res.results[i]["out"].reshape(TPC, outw) for i in range(NCORES)], axis=0
    )


def kernel(logits, targets, step_count):
    logits = np.asarray(logits, dtype=np.float32)
    targets = np.asarray(targets).astype(np.int64)
    step = int(np.asarray(step_count))

    lf = logits.reshape(N, V)
    tf = targets.reshape(N)

    cw, ncc, outw = _v2_layout()
    stats = _device_stats(lf, outw)
    m8 = stats[:, :8 * ncc].astype(np.float64)        # 8 max-partials per chunk
    se_parts = stats[:, 8 * ncc:9 * ncc].astype(np.float64)
    sx_parts = stats[:, 9 * ncc:10 * ncc].astype(np.float64)
    se = se_parts.sum(axis=1)
    # sum(e*l): exact STT partials on STT slots, finite-difference of the
    # two exp sums on FD slots (slot = (pt, chunk), same for every core)
    flags = np.array(_v3_stt_flags(ncc, V3_STT_NUM, V3_STT_MODE)).reshape(NPT, ncc)
    fdm = ~flags[(np.arange(N) % TPC) // P]           # (N, ncc) True = FD slot
    sel = np.where(fdm, (sx_parts - se_parts) / DELTA, sx_parts).sum(axis=1)

    # top-2 of e per token: per-chunk maxes of e (bf16) from the device;
    # exact l-space rescan of the argmax chunk for the within-chunk top-2.
    me = m8.reshape(N, ncc, 8).max(axis=2)            # (N, ncc) chunk maxes
    cstar = np.argmax(me, axis=1)
    base = cstar * cw
    idx = base[:, None] + np.arange(cw)[None, :]
    valid = idx < V
    win = lf[np.arange(N)[:, None], np.minimum(idx, V - 1)].astype(np.float64)
    win = np.where(valid, win, -np.inf)
    wtop = np.partition(win, -2, axis=1)[:, -2:]
    e_w1 = np.exp(wtop[:, 1])
    e_w2 = np.exp(wtop[:, 0])
    me_rest = me.copy()
    me_rest[np.arange(N), cstar] = -np.inf
    cross = me_rest.max(axis=1)
    e1 = np.maximum(e_w1, cross)
    e2 = np.maximum(np.minimum(e_w1, cross), e_w2)

    # epilogue in f64 (mirrors reference formulas)
    log_v = np.log(np.float32(V)).astype(np.float64)
    lse = np.log(se)
    l_tgt = lf[np.arange(N), tf].astype(np.float64)
    loss = lse - l_tgt                                 # -logp[target]
    p1 = e1 / se                                       # confidence
    p2 = e2 / se
    margin = p1 - p2
    entropy = lse - sel / se                           # -sum p*logp
    difficulty = (entropy / log_v + (1.0 - margin) + loss / log_v) / 3.0

    progress = min(1.0, float(step) / max(1, WARMUP_STEPS))
    base_ratio = 1.0 - progress * (1.0 - MIN_TOKENS_RATIO)
    mean_conf = p1.mean()
    ratio = np.clip(
        base_ratio * (1.0 + THRESHOLD_SENSITIVITY * (0.5 - mean_conf)), 0.05, 1.0
    )
    k = int(np.clip(np.round(ratio * N), 1, N))
    thresh = np.sort(difficulty)[::-1][k - 1]
    mask = (difficulty >= thresh).astype(np.float64)
    tokens_selected = mask.sum()
    out = (loss * mask).sum() / max(tokens_selected, 1.0)
    return np.asarray(out, dtype=np.float32)

